# revision 28
# baseline (speedup 1.0000x reference)
"""Trainium2 Bass kernel for nn_ClearMeshLoss (8-core SPMD), v2.

Chamfer/normal-consistency strategy (single distance-matrix pass):
  - Each core owns a 1250-row block of pred_points (padded to 1280 = 10
    strips of 128) and all 10000 gt_points (padded to 10240 cols).
  - A K=13 fp16-split augmented matmul produces -d_ij = 2 a.b - |a|^2 -
    |b|^2 directly in PSUM at fp32 accuracy (fp16 hi/lo splits make every
    product exact; only the al*bl cross term ~1e-7 is dropped).  1
    cycle/row on the PE vs 4 for fp32.
  - ACT copies each PSUM supertile to an SBUF fp16 strip (values are -d,
    so fp16's relative precision applies to the distance itself).
  - DVE work runs in 2x/4x fp16 perf modes: a block-fold tree gives
    per-128-block row maxes -> row max (A-side min) + winning block w;
    an in-place is_ge mask (4x) + a fold over the 80 blocks gives the
    within-block argmax position; a running elementwise max folds strips
    into the B-side per-column max.
  - B-side column max over the 128 partitions: PE transposes 128x128
    blocks into PSUM (one per 2KB bank) and DVE reduces them.
  - Host combines: minA = -rowmax, idx = 128*w + loc, minB = -colmax
    (max over cores), normal gather + cosine on host.
sdf/eikonal: data-parallel over the 200000 elements (25000/core).
edge loss: host does integer edge pairing and the cheap vector diffs +
  cross products; device does dots, norms, cos, relu and the sum.
watertight: integer-only, host.
"""
import numpy as np

# ---------------------------------------------------------------- constants
SDF_W, EIK_W, CH_W, NORM_W, EDGE_W, WT_W = 1.0, 0.1, 1.0, 0.5, 0.3, 0.2
TRUNC, SURF_W, DIH_THR = 0.1, 5.0, 0.5
SIGMA = TRUNC / 3.0

N_CORES = 8
FAR = 100.0

FULL_CFG = dict(
    npts=10000,
    rows_pad=1280,       # per-core padded row count (10 strips of 128)
    cols_pad=10240,      # padded column count
    super_w=2048,        # PSUM supertile width (4 banks)
    tile_w=512,          # matmul free dim
    kdim=13,             # fp16-split augmented contraction size
    sdf_n=200000,
    sdf_shard=25000,
    sdf_f=196,
    eik_f=196,
    pair_cap=122880,     # 8*128*120 edge pairs on device
    pair_f=120,
)

_PROG_CACHE = {}


def build_program(cfg, phases=("cham", "sdf", "eik", "edge")):
    from contextlib import ExitStack
    import concourse.bacc as bacc
    import concourse.bass as bass
    import concourse.tile as tile
    from concourse import mybir

    f32 = mybir.dt.float32
    f16 = mybir.dt.float16
    AX = mybir.AxisListType
    OP = mybir.AluOpType
    AF = mybir.ActivationFunctionType

    rows_pad = cfg["rows_pad"]
    cols_pad = cfg["cols_pad"]
    super_w = cfg["super_w"]
    tile_w = cfg["tile_w"]
    kdim = cfg["kdim"]
    sdf_f = cfg["sdf_f"]
    eik_f = cfg["eik_f"]
    pair_f = cfg["pair_f"]

    n_strips = rows_pad // 128          # 10
    n_super = cols_pad // super_w       # 5
    mm_per_super = super_w // tile_w    # 4
    n_blocks = cols_pad // 128          # 80

    nc = bacc.Bacc("TRN2", target_bir_lowering=False)

    # ---- inputs ----
    d_lhs = nc.dram_tensor("lhs_pack", [kdim, rows_pad], f16, kind="ExternalInput")
    d_rhs = nc.dram_tensor("rhs_pack", [kdim, cols_pad], f16, kind="ExternalInput")
    d_ident = nc.dram_tensor("ident", [128, 128], f16, kind="ExternalInput")
    d_sdf_pred = nc.dram_tensor("sdf_pred", [128, sdf_f], f32, kind="ExternalInput")
    d_sdf_gt = nc.dram_tensor("sdf_gt", [128, sdf_f], f32, kind="ExternalInput")
    d_eik_pred = nc.dram_tensor("eik_pred", [128 * eik_f + 1], f32, kind="ExternalInput")
    d_eik_gt = nc.dram_tensor("eik_gt", [128, eik_f], f32, kind="ExternalInput")
    # edge planes: naP1(3), nbP1(3) face normals (unnormalized), see host prep
    d_edge = nc.dram_tensor("edge_in", [128, 6, pair_f], f32, kind="ExternalInput")

    # ---- outputs ----
    # cham cols: [0..10) rowmax(-d); [10..20) w; [20..90) argmax path bits
    # (7 per strip, weight 64,32,16,8,4,2,1)
    d_cham = nc.dram_tensor("cham_out", [128, 9 * n_strips], f32, kind="ExternalOutput")
    d_bcol = nc.dram_tensor("bcol_out", [128, n_blocks], f32, kind="ExternalOutput")
    # part cols: 0 sdf_absdiff, 1 sdf_4e_absdiff, 2 eik_num, 3 eik_cnt, 4 edge_relu
    d_part = nc.dram_tensor("part_out", [128, 8], f32, kind="ExternalOutput")

    with tile.TileContext(nc) as tc, ExitStack() as octx:
        singles = octx.enter_context(tc.tile_pool(name="singles", bufs=1))
        cham_o = singles.tile([128, 9 * n_strips], f32)
        bcol_o = singles.tile([128, n_blocks], f32)
        part_o = singles.tile([128, 8], f32)
        nc.vector.memset(part_o, 0.0)

        # ---- sdf / eikonal / edge first: their DVE work fills the
        # pipeline-fill window while the first strip's matmuls+copies run.
        # Pools stay open (outer octx) so SBUF space isn't reused by the
        # chamfer pools, which would serialize the phases. ----
        spool = octx.enter_context(tc.tile_pool(name="spool", bufs=1))
        if "sdf" not in phases:
            nc.vector.memset(part_o[:, 0:2], 0.0)
        if "eik" not in phases:
            nc.vector.memset(part_o[:, 2:3], 0.0)
            nc.vector.memset(part_o[:, 3:4], 1.0)
        if "sdf" in phases:
            _emit_sdf(nc, spool, part_o, d_sdf_pred, d_sdf_gt, sdf_f,
                      f32, AX, OP, AF)
        if "eik" in phases:
            _emit_eik(nc, bass, spool, part_o, d_eik_pred, d_eik_gt, eik_f,
                      f32, AX, OP, AF)
        if "edge" not in phases:
            nc.vector.memset(part_o[:, 4:5], 0.0)
        if "edge" in phases:
            epool = octx.enter_context(tc.tile_pool(name="epool", bufs=1))
            _emit_edge(nc, epool, part_o, d_edge, pair_f, f32, AX, OP, AF)

        if "cham" not in phases:
            nc.vector.memset(cham_o, 0.0)
            nc.vector.memset(bcol_o, 0.0)
        if "cham" in phases:
          with ExitStack() as ctx:
            cpool = ctx.enter_context(tc.tile_pool(name="cpool", bufs=1))
            strips = ctx.enter_context(tc.tile_pool(name="strips", bufs=3))
            fpool = ctx.enter_context(tc.tile_pool(name="fpool", bufs=2))
            mmctx = ctx.enter_context(ExitStack())
            psum = mmctx.enter_context(tc.tile_pool(name="psum", bufs=2, space="PSUM"))

            rhs_t = cpool.tile([kdim, cols_pad], f16)
            for c in range(n_super):  # chunked: first matmul starts sooner
                nc.sync.dma_start(out=rhs_t[:, c * super_w:(c + 1) * super_w],
                                  in_=d_rhs[:, c * super_w:(c + 1) * super_w])
            lhs_t = cpool.tile([kdim, rows_pad], f16)
            nc.sync.dma_start(out=lhs_t, in_=d_lhs[:, :])
            ident_t = cpool.tile([128, 128], f16)
            nc.sync.dma_start(out=ident_t, in_=d_ident[:, :])

            iota80 = cpool.tile([128, n_blocks], f16)
            nc.gpsimd.iota(out=iota80[:, :], pattern=[[1, n_blocks]], base=0,
                           channel_multiplier=0,
                           allow_small_or_imprecise_dtypes=True)

            runB = [cpool.tile([128, cols_pad], f16, name=f"runB{i}")
                    for i in range(2)]

            for s in range(n_strips):
                strip = strips.tile([128, cols_pad], f16, tag="strip")
                for c in range(n_super):
                    ps = psum.tile([128, super_w], f32, tag="ps")
                    for m in range(mm_per_super):
                        lo = c * super_w + m * tile_w
                        nc.tensor.matmul(ps[:, m * tile_w:(m + 1) * tile_w],
                                         lhs_t[:, s * 128:(s + 1) * 128],
                                         rhs_t[:, lo:lo + tile_w],
                                         start=True, stop=True)
                    nc.scalar.activation(
                        out=strip[:, c * super_w:(c + 1) * super_w],
                        in_=ps[:, :], func=AF.Copy)

                # ---- A-side: block-fold tree -> blockmax [128, 80] ----
                v = strip[:, :].rearrange("p (b k) -> p b k", k=128)
                f1 = fpool.tile([128, n_blocks * 64], f16, tag="f1")
                nc.vector.tensor_tensor(
                    out=f1[:, :].rearrange("p (b k) -> p b k", k=64),
                    in0=v[:, :, 0:64], in1=v[:, :, 64:128], op=OP.max)
                tiers = [(strip, 128), (f1, 64)]
                cur, width = f1, 64
                while width > 1:
                    half = width // 2
                    nxt = fpool.tile([128, n_blocks * half], f16, tag=f"fw{half}")
                    cv = cur[:, :].rearrange("p (b k) -> p b k", k=width)
                    nc.vector.tensor_tensor(
                        out=nxt[:, :].rearrange("p (b k) -> p b k", k=half),
                        in0=cv[:, :, 0:half], in1=cv[:, :, half:width], op=OP.max)
                    cur, width = nxt, half
                    tiers.append((cur, width))
                blockmax = cur  # [128, 80] f16

                rmax = cham_o[:, s:s + 1]
                nc.vector.tensor_reduce(out=rmax, in_=blockmax[:, :], axis=AX.X,
                                        op=OP.max)
                junk80 = fpool.tile([128, n_blocks], f16, tag="junk80")
                nc.vector.scalar_tensor_tensor(
                    out=junk80, in0=blockmax[:, :], scalar=rmax,
                    in1=iota80[:, :], op0=OP.is_ge, op1=OP.mult,
                    accum_out=cham_o[:, n_strips + s:n_strips + s + 1])

                # ---- B-side running column max ----
                if s == 0:
                    nc.scalar.activation(out=runB[0][:, :], in_=strip[:, :],
                                         func=AF.Copy)
                else:
                    nc.vector.tensor_tensor(out=runB[s % 2][:, :],
                                            in0=strip[:, :],
                                            in1=runB[(s + 1) % 2][:, :],
                                            op=OP.max)

                # ---- argmax path bits: is_ge+accum on each tier's right
                # half (binary descent; tie contamination is negligible) ----
                for kbit in range(1, 8):
                    src, w_prev = tiers[kbit - 1][0], tiers[kbit - 1][1]
                    half = w_prev // 2
                    sv = src[:, 0:n_blocks * w_prev].rearrange(
                        "p (b k) -> p b k", k=w_prev)
                    col = 2 * n_strips + s * 7 + (kbit - 1)
                    nc.vector.tensor_scalar(
                        out=sv[:, :, half:w_prev], in0=sv[:, :, half:w_prev],
                        scalar1=rmax, scalar2=0.0, op0=OP.is_ge, op1=OP.add,
                        accum_out=cham_o[:, col:col + 1])

            # ---- B-side final: transpose blocks + reduce over rows ----
            mmctx.close()  # release matmul PSUM banks
            runB_fin = runB[(n_strips - 1) % 2]
            with ExitStack() as bctx:
                psum_t = bctx.enter_context(
                    tc.tile_pool(name="psum_t", bufs=2, space="PSUM"))
                for g in range(n_blocks // 4):
                    pt = psum_t.tile([128, 4, 1024], f16, tag="pt")
                    for b in range(4):
                        j0 = (g * 4 + b) * 128
                        nc.tensor.transpose(pt[:, b, 0:128],
                                            runB_fin[:, j0:j0 + 128],
                                            ident_t[:, :])
                    nc.vector.tensor_reduce(out=bcol_o[:, g * 4:(g + 1) * 4],
                                            in_=pt[:, :, 0:128], axis=AX.X,
                                            op=OP.max)

        nc.sync.dma_start(out=d_cham[:, :], in_=cham_o[:, :])
        nc.sync.dma_start(out=d_bcol[:, :], in_=bcol_o[:, :])
        nc.sync.dma_start(out=d_part[:, :], in_=part_o[:, :])

    nc.compile()
    return nc


def _emit_sdf(nc, spool, part_o, d_sdf_pred, d_sdf_gt, sdf_f, f32, AX, OP, AF):
    pr = spool.tile([128, sdf_f], f32)
    g = spool.tile([128, sdf_f], f32)
    nc.sync.dma_start(out=pr, in_=d_sdf_pred[:, :])
    nc.sync.dma_start(out=g, in_=d_sdf_gt[:, :])

    prc = spool.tile([128, sdf_f], f32)
    gc = spool.tile([128, sdf_f], f32)
    nc.vector.tensor_scalar(out=prc, in0=pr, scalar1=TRUNC, scalar2=-TRUNC,
                            op0=OP.min, op1=OP.max)
    nc.vector.tensor_scalar(out=gc, in0=g, scalar1=TRUNC, scalar2=-TRUNC,
                            op0=OP.min, op1=OP.max)
    diff = spool.tile([128, sdf_f], f32)
    nc.vector.tensor_tensor(out=diff, in0=prc, in1=gc, op=OP.subtract)
    absdiff = spool.tile([128, sdf_f], f32)
    nc.scalar.activation(out=absdiff, in_=diff, func=AF.Abs)
    nc.vector.tensor_reduce(out=part_o[:, 0:1], in_=absdiff, axis=AX.X,
                            op=OP.add)
    absg = spool.tile([128, sdf_f], f32)
    nc.scalar.activation(out=absg, in_=gc, func=AF.Abs)
    e = spool.tile([128, sdf_f], f32)
    nc.scalar.activation(out=e, in_=absg, func=AF.Exp, scale=-1.0 / SIGMA)
    dead = spool.tile([128, sdf_f], f32)
    nc.vector.scalar_tensor_tensor(out=dead, in0=e, scalar=SURF_W - 1.0,
                                   in1=absdiff, op0=OP.mult, op1=OP.mult,
                                   accum_out=part_o[:, 1:2])


def _emit_eik(nc, bass, spool, part_o, d_eik_pred, d_eik_gt, eik_f, f32, AX, OP, AF):
    ep0 = spool.tile([128, eik_f], f32)
    ep1 = spool.tile([128, eik_f], f32)
    base = d_eik_pred[:]
    src0 = bass.AP(tensor=base.tensor, offset=0, ap=[[eik_f, 128], [1, eik_f]])
    src1 = bass.AP(tensor=base.tensor, offset=1, ap=[[eik_f, 128], [1, eik_f]])
    nc.sync.dma_start(out=ep0[:, :], in_=src0)
    nc.sync.dma_start(out=ep1[:, :], in_=src1)
    eg = spool.tile([128, eik_f], f32)
    nc.sync.dma_start(out=eg, in_=d_eik_gt[:, :])

    dx = spool.tile([128, eik_f], f32)
    nc.vector.tensor_tensor(out=dx, in0=ep1[:, :], in1=ep0[:, :], op=OP.subtract)
    absdx = spool.tile([128, eik_f], f32)
    nc.scalar.activation(out=absdx, in_=dx, func=AF.Abs)
    t = spool.tile([128, eik_f], f32)
    nc.vector.tensor_scalar(out=t, in0=absdx, scalar1=-1.0, scalar2=None,
                            op0=OP.add)
    t2 = spool.tile([128, eik_f], f32)
    nc.vector.tensor_tensor(out=t2, in0=t, in1=t, op=OP.mult)
    abseg = spool.tile([128, eik_f], f32)
    nc.scalar.activation(out=abseg, in_=eg, func=AF.Abs)
    mask = spool.tile([128, eik_f], f32)
    nc.vector.tensor_scalar(out=mask, in0=abseg, scalar1=TRUNC, scalar2=None,
                            op0=OP.is_lt)
    mt2 = spool.tile([128, eik_f], f32)
    nc.vector.tensor_tensor(out=mt2, in0=t2, in1=mask, op=OP.mult)
    nc.vector.tensor_reduce(out=part_o[:, 2:3], in_=mt2, axis=AX.X, op=OP.add)
    nc.vector.tensor_reduce(out=part_o[:, 3:4], in_=mask, axis=AX.X, op=OP.add)


def _emit_edge(nc, epool, part_o, d_edge, pair_f, f32, AX, OP, AF):
    # device gets unnormalized face normals na (planes 0-2), nb (planes 3-5)
    ev = epool.tile([128, 6, pair_f], f32)
    nc.sync.dma_start(out=ev[:, :, :], in_=d_edge[:, :, :])

    na = ev[:, 0:3, :]
    nb = ev[:, 3:6, :]

    def dot3(a, b, nm):
        prod = epool.tile([128, 3, pair_f], f32, name=f"prod_{nm}")
        nc.vector.tensor_tensor(out=prod, in0=a, in1=b, op=OP.mult)
        t = epool.tile([128, pair_f], f32, name=f"dt_{nm}")
        nc.vector.tensor_tensor(out=t, in0=prod[:, 0, :], in1=prod[:, 1, :],
                                op=OP.add)
        d = epool.tile([128, pair_f], f32, name=f"dot_{nm}")
        nc.vector.tensor_tensor(out=d, in0=t, in1=prod[:, 2, :], op=OP.add)
        return d

    dot = dot3(na, nb, "ab")
    na2 = dot3(na, na, "aa")
    nb2 = dot3(nb, nb, "bb")
    prod2 = epool.tile([128, pair_f], f32)
    nc.vector.tensor_tensor(out=prod2, in0=na2, in1=nb2, op=OP.mult)
    sa = epool.tile([128, pair_f], f32)
    nc.scalar.activation(out=sa, in_=prod2, func=AF.Sqrt)
    sac = epool.tile([128, pair_f], f32)
    nc.vector.tensor_scalar(out=sac, in0=sa, scalar1=1e-24, scalar2=None,
                            op0=OP.max)
    rs = epool.tile([128, pair_f], f32)
    nc.vector.reciprocal(out=rs, in_=sac)
    cos = epool.tile([128, pair_f], f32)
    nc.vector.tensor_tensor(out=cos, in0=dot, in1=rs, op=OP.mult)
    relu = epool.tile([128, pair_f], f32)
    nbias = epool.tile([128, 1], f32)
    nc.vector.memset(nbias, -DIH_THR)
    nc.scalar.activation(out=relu, in_=cos, func=AF.Relu, bias=nbias[:, 0:1],
                         accum_out=part_o[:, 4:5])


def get_program(cfg_key="full"):
    if cfg_key not in _PROG_CACHE:
        _PROG_CACHE[cfg_key] = build_program(FULL_CFG)
    return _PROG_CACHE[cfg_key]


# ================================================================== host side
def _split16(x64):
    """fp16 hi/lo split of a float64 array."""
    hi = x64.astype(np.float16)
    lo = (x64 - hi.astype(np.float64)).astype(np.float16)
    return hi, lo


def _host_prep(inputs, cfg):
    np_f32 = np.float32
    pred_pts = np.asarray(inputs["pred_points"][0], dtype=np.float64)  # [N,3]
    gt_pts = np.asarray(inputs["gt_points"][0], dtype=np.float64)
    npts = cfg["npts"]
    rows_pad, cols_pad = cfg["rows_pad"], cfg["cols_pad"]
    shard = npts // N_CORES

    def pad_pts(p, n):
        out = np.full((n, 3), FAR, np.float64)
        out[:p.shape[0]] = p
        return out

    # rhs pack (shared): [13, cols_pad] f16
    b = pad_pts(gt_pts, cols_pad)
    bh, bl = _split16(b)
    q2 = (b * b).sum(-1)
    q2h, q2l = _split16(q2)
    ones_c = np.ones(cols_pad, np.float16)
    rhs_pack = np.ascontiguousarray(np.concatenate([
        (2.0 * bh.astype(np.float64)).astype(np.float16).T,
        (2.0 * bl.astype(np.float64)).astype(np.float16).T,
        (2.0 * bh.astype(np.float64)).astype(np.float16).T,
        (-ones_c)[None, :], (-ones_c)[None, :],
        (-q2h)[None, :], (-q2l)[None, :],
    ], 0))
    ident = np.eye(128, dtype=np.float16)

    # --- sdf / eikonal shards ---
    pred_sdf = np.asarray(inputs["pred_sdf"]).reshape(-1).astype(np_f32)
    gt_sdf = np.asarray(inputs["gt_sdf"]).reshape(-1).astype(np_f32)
    n_tot = pred_sdf.shape[0]
    sdf_shard, sdf_f, eik_f = cfg["sdf_shard"], cfg["sdf_f"], cfg["eik_f"]
    n_batch = inputs["pred_sdf"].shape[1]

    # --- edge pairing on host (int32 faces), cross products on host ---
    verts = np.asarray(inputs["extracted_vertices"], dtype=np_f32)
    faces = np.asarray(inputs["extracted_faces"], dtype=np.int64)
    V = verts.shape[0]
    Fn = faces.shape[0]
    a_ = faces
    b_ = np.roll(faces, -1, axis=1)
    lo = np.minimum(a_, b_)
    hi = np.maximum(a_, b_)
    key = (lo * V + hi).reshape(-1)
    fid = np.repeat(np.arange(Fn, dtype=np.int64), 3)
    order = np.argsort(key, kind="stable")
    k = key[order]
    f = fid[order]
    same_next = k[:-1] == k[1:]
    prev = np.concatenate([[False], same_next[:-1]])
    nxt = np.concatenate([same_next[1:], [False]])
    is_pair = same_next & ~prev & ~nxt
    pos = np.nonzero(is_pair)[0]
    fa = f[pos]
    fb = f[pos + 1]
    npairs = int(pos.shape[0])
    is_start = np.concatenate([[True], k[1:] != k[:-1]])
    starts = np.nonzero(is_start)[0]
    run_len = np.diff(np.concatenate([starts, [k.shape[0]]]))
    total_unique = int(starts.shape[0])
    bad = int((run_len != 2).sum())
    wt = (bad / total_unique) if total_unique > 0 else 0.0

    pair_cap = cfg["pair_cap"]
    n_dev = min(npairs, pair_cap)
    planes = np.zeros((6, pair_cap), np_f32)
    if n_dev > 0:
        va = verts[faces[fa[:n_dev]]].astype(np.float64)  # [n,3vert,3xyz]
        vb = verts[faces[fb[:n_dev]]].astype(np.float64)
        na = np.cross(va[:, 1] - va[:, 0], va[:, 2] - va[:, 0])
        nb = np.cross(vb[:, 1] - vb[:, 0], vb[:, 2] - vb[:, 0])
        planes[0:3, :n_dev] = na.T
        planes[3:6, :n_dev] = nb.T
    edge_extra = 0.0
    if npairs > pair_cap:
        va = verts[faces[fa[pair_cap:]]].astype(np.float64)
        vb = verts[faces[fb[pair_cap:]]].astype(np.float64)
        na = np.cross(va[:, 1] - va[:, 0], va[:, 2] - va[:, 0])
        nb = np.cross(vb[:, 1] - vb[:, 0], vb[:, 2] - vb[:, 0])
        na /= np.maximum(np.linalg.norm(na, axis=-1, keepdims=True), 1e-12)
        nb /= np.maximum(np.linalg.norm(nb, axis=-1, keepdims=True), 1e-12)
        cos = (na * nb).sum(-1)
        edge_extra = float(np.maximum(cos - DIH_THR, 0.0).sum())

    pair_f = cfg["pair_f"]
    planes8 = planes.reshape(6, N_CORES, 128, pair_f).transpose(1, 2, 0, 3)
    planes8 = np.ascontiguousarray(planes8)  # [N_CORES, 128, 6, pair_f]

    in_maps = []
    for c in range(N_CORES):
        a = pad_pts(pred_pts[c * shard:(c + 1) * shard], rows_pad)
        ah, al = _split16(a)
        p2 = (a * a).sum(-1)
        p2h, p2l = _split16(p2)
        ones_r = np.ones(rows_pad, np.float16)
        lhs_pack = np.ascontiguousarray(np.concatenate([
            ah.T, ah.T, al.T,
            p2h[None, :], p2l[None, :],
            ones_r[None, :], ones_r[None, :],
        ], 0))

        sp = np.zeros(128 * sdf_f, np_f32)
        sg = np.zeros(128 * sdf_f, np_f32)
        sl = pred_sdf[c * sdf_shard:(c + 1) * sdf_shard]
        sp[:sl.shape[0]] = sl
        sg[:sl.shape[0]] = gt_sdf[c * sdf_shard:(c + 1) * sdf_shard]

        ep = np.zeros(128 * eik_f + 1, np_f32)
        src = pred_sdf[c * sdf_shard: c * sdf_shard + 128 * eik_f + 1]
        ep[:src.shape[0]] = src
        eg = np.full(128 * eik_f, 1e9, np_f32)
        gsrc = gt_sdf[c * sdf_shard: c * sdf_shard + 128 * eik_f]
        eg[:gsrc.shape[0]] = gsrc
        locs = np.arange(128 * eik_f)
        glob = locs + c * sdf_shard
        bad_m = (locs >= sdf_shard) | ((glob % n_batch) == n_batch - 1) | \
                (glob >= n_tot - 1)
        eg[bad_m] = 1e9

        in_maps.append({
            "lhs_pack": lhs_pack,
            "rhs_pack": rhs_pack,
            "ident": ident,
            "sdf_pred": sp.reshape(128, sdf_f),
            "sdf_gt": sg.reshape(128, sdf_f),
            "eik_pred": ep,
            "eik_gt": eg.reshape(128, eik_f),
            "edge_in": np.ascontiguousarray(planes8[c]),
        })

    meta = dict(npairs=npairs, wt=wt, edge_extra=edge_extra, shard=shard)
    return in_maps, meta


def _host_post(inputs, cfg, results, meta):
    npts = cfg["npts"]
    shard = meta["shard"]
    rows_pad = cfg["rows_pad"]
    cols_pad = cfg["cols_pad"]
    n_strips = rows_pad // 128
    n_blocks = cols_pad // 128

    rowmax = np.empty(npts, np.float64)
    idxA = np.empty(npts, np.int64)
    bcol_all = np.empty((N_CORES, 128, n_blocks), np.float64)
    bit_w = np.array([64, 32, 16, 8, 4, 2, 1], np.float64)
    for c in range(N_CORES):
        cham = results[c]["cham_out"]  # [128, 90]
        rm = cham[:, 0:n_strips].T.reshape(-1)[:shard]
        w = cham[:, n_strips:2 * n_strips].T.reshape(-1)[:shard]
        bits = cham[:, 2 * n_strips:9 * n_strips].reshape(128, n_strips, 7)
        # clip each bit to {0,1}: exact fp16 ties can make a count >1
        loc = (np.minimum(bits.astype(np.float64), 1.0) *
               bit_w[None, None, :]).sum(-1)            # [128, n_strips]
        loc = loc.T.reshape(-1)[:shard]
        rowmax[c * shard:(c + 1) * shard] = rm
        idxA[c * shard:(c + 1) * shard] = (
            128.0 * w.astype(np.float64) + loc).astype(np.int64)
        bcol_all[c] = results[c]["bcol_out"]

    minA = -rowmax
    # bcol_all[c, jj, b] = colmax of col j = b*128 + jj over core c's rows
    colmax = bcol_all.max(axis=0)                       # [128, n_blocks]
    minB = -colmax.T.reshape(-1)[:npts]
    ch = minA.mean() + minB.mean()

    pn = np.asarray(inputs["pred_normals"][0], dtype=np.float64)
    gn = np.asarray(inputs["gt_normals"][0], dtype=np.float64)
    idxA = np.clip(idxA, 0, npts - 1)
    matched = gn[idxA]
    eps = 1e-8
    num = (pn * matched).sum(-1)
    den = np.maximum(np.linalg.norm(pn, axis=-1), eps) * \
        np.maximum(np.linalg.norm(matched, axis=-1), eps)
    nrm = float(np.mean(1.0 - np.abs(num / den)))

    parts = np.stack([results[c]["part_out"] for c in range(N_CORES)])
    psum = parts.astype(np.float64).sum(axis=(0, 1))
    sdf = (psum[0] + psum[1]) / float(cfg["sdf_n"])
    eik = (psum[2] / psum[3]) if psum[3] > 0 else 0.0

    npairs = meta["npairs"]
    edge = ((psum[4] + meta["edge_extra"]) / npairs) if npairs > 0 else 0.0

    total = (SDF_W * sdf + EIK_W * eik + CH_W * ch + NORM_W * nrm +
             EDGE_W * edge + WT_W * meta["wt"])
    return np.asarray(np.float32(total))


def kernel(**inputs):
    from concourse.bass_utils import run_bass_kernel_spmd
    cfg = FULL_CFG
    nc = get_program()
    in_maps, meta = _host_prep(inputs, cfg)
    res = run_bass_kernel_spmd(nc, in_maps, core_ids=list(range(N_CORES)))
    return _host_post(inputs, cfg, res.results, meta)
